# revision 3
# baseline (speedup 1.0000x reference)
"""DispersionLoss kernel for Trainium2 (8 NeuronCores, Bass/Tile).

Reference computation (N=16384, F=64, K=32, C=128):
    bin_mass[f,k]  = sum_n m[n,f,k] + EPS
    SWY[f,k,c]     = sum_n m[n,f,k] * y[n,c]
    cent[f,k,c]    = SWY / bin_mass
    loss_dispersion= sum_fk (A/bin_mass - c_sq)   [EPS*c_sq/bin_mass ~1e-11, dropped]
        where A[f,k] = sum_n m[n,f,k]*|y_n|^2
    loss_entropy   = sum_fk p*log(p+EPS), p = bin_mass/N  (host, from shipped bin_mass)
    loss_repulsion = sum_f sum_k exp(-|cent[f,k]-cent[f,k+1]|^2)
    loss_inter     = sum_f (sum_{kj} exp(-pairwise) - K) / 2 / F

Sharding: over F (8 features per core) -> every loss term decomposes per-f,
no cross-core collectives; host sums 8 partial vectors.

v2 design (vs the 41.5us single-queue baseline):
  - inputs quantized to fp8 e4m3 on host; ysq shipped as fp8 hi+lo pair.
  - g repacked into TWO bin-half slabs (h=0: bins 0..127 = features 0..3,
    h=1: bins 128..255).  All of slab0 streams before slab1, so the h=0
    half finishes its PSUM accumulation at the DMA midpoint and its FULL
    tail (centroids, pairwise exp, repulsion) runs overlapped with slab1's
    DMA + matmuls.  Post-DMA critical path = h=1 half-tail only.
  - input DMA split across both HWDGE queues: y (4 chunks) on the scalar
    queue, g (8 x 512KB blocks) on the sync queue -> parallel descriptor
    issue, earlier first byte, fewer pacing gaps.
  - no Ln on device: bin_mass (128,2) is DMA'd out and the tiny (F,K)
    entropy term is computed on host in f64 (exactly like _finalize
    already sums partials).  Scalar engine touches ONLY Copy/Identity/Exp
    -> the Exp table is loaded once, zero table reloads in the tail.
  - per-half pairwise (128x128, within-half; halves never share a feature)
    with the -B cross-feature block bias so exp() zeroes them and the Exp
    ACT's accum_out yields the inter-loss block sums for free.
  - HAM ramp absorbed by ~16 warm matmuls (~3.4us of cold PE time); no
    dummy matmuls needed elsewhere (PE never idles >5us before the end).
"""

import numpy as np

N = 16384
F = 64
K = 32
C = 128
NCORES = 8
F_PER_CORE = F // NCORES          # 8
FK = F_PER_CORE * K               # 256 bins per core
NPAIR = N // 256                  # 64 subtile pairs (DoubleRow: 256 rows/mm)
W = 132                           # moving cols: [y(128) | 1 | ysq_h | ysq_l | pad]
PPB = 16                          # pairs per g block
NBLK = NPAIR // PPB               # 4 blocks per slab
CSC = 16.0                        # centered-centroid scale (keeps fp16 normal)
BBIAS = 3840.0                    # cross-feature psE bias: exp arg -= 30
NWARM = 16                        # PE clock warm-up matmuls (~3.4us cold)

LAMBDA_ENTROPY = 0.1
LAMBDA_REPULSION = 0.5
LAMBDA_INTER = 0.3
EPS = 1e-8

_NC_CACHE = {}


def _f8dtype():
    import ml_dtypes
    return ml_dtypes.float8_e4m3


def _pack_g(gc: np.ndarray) -> np.ndarray:
    """(N, FK) fp8 -> (8*128, PPB*2*128): slab h (4 blocks), block row p holds,
    for the 16 pairs u of the block, [i=0 | i=1] x 128 half-bins where the
    n-row is 256*u + 128*i + p."""
    x = gc.reshape(NPAIR, 2, 128, 2, 128)           # u, i, p, h, fk
    x = x.reshape(NBLK, PPB, 2, 128, 2, 128)        # blk, ul, i, p, h, fk
    x = x.transpose(4, 0, 3, 1, 2, 5)               # h, blk, p, ul, i, fk
    return np.ascontiguousarray(x.reshape(2 * NBLK * 128, PPB * 2 * 128))


def _pack_y(yslab: np.ndarray) -> np.ndarray:
    """(N, W) fp8 -> (128, NPAIR*2*W): partition p holds pair-major slabs."""
    return np.ascontiguousarray(
        yslab.reshape(NPAIR, 2, 128, W).transpose(2, 0, 1, 3).reshape(128, NPAIR * 2 * W)
    )


def _finalize(parts: np.ndarray, masses: np.ndarray):
    """parts: (ncores, 8) = [wv0, wv1, eall0, eall1, rep0, rep1, rx0, rx1].
    masses: (ncores, 128, 2) = bin_mass (+EPS) per half."""
    r = parts.astype(np.float64).sum(axis=0)
    disp = r[0] + r[1]
    p = masses.astype(np.float64).reshape(-1) / N
    ent = float(np.sum(p * np.log(p + EPS)))
    rep = (r[4] + r[5]) - (r[6] + r[7])
    inter = (r[2] + r[3] - F * K) / (2.0 * F)
    tot = disp + LAMBDA_ENTROPY * ent + LAMBDA_REPULSION * rep + LAMBDA_INTER * inter
    return tuple(np.float32(v) for v in (tot, disp, ent, rep, inter))


def _build_nc():
    import concourse.bacc as bacc
    import concourse.tile as tile
    from concourse import mybir

    f32 = mybir.dt.float32
    f16 = mybir.dt.float16
    f8 = mybir.dt.float8e4
    DR = mybir.MatmulPerfMode.DoubleRow
    AF = mybir.ActivationFunctionType
    OP = mybir.AluOpType

    nc = bacc.Bacc("TRN2", target_bir_lowering=False, debug=False,
                   enable_asserts=False, enable_partition_id=False)
    g_dram = nc.dram_tensor("g", (2 * NBLK * 128, PPB * 2 * 128), f8,
                            kind="ExternalInput").ap()
    y_dram = nc.dram_tensor("y", (128, NPAIR * 2 * W), f8, kind="ExternalInput").ap()
    out_dram = nc.dram_tensor("out", (1, 8), f32, kind="ExternalOutput").ap()
    mass_dram = nc.dram_tensor("mass", (128, 2), f32, kind="ExternalOutput").ap()

    with tile.TileContext(nc) as tc:
        with (
            tc.tile_pool(name="singles", bufs=1) as singles,
            tc.tile_pool(name="gpool", bufs=8) as gpool,
            tc.tile_pool(name="scr", bufs=2) as scr,
            tc.tile_pool(name="ph2", bufs=1) as ph2,
            tc.tile_pool(name="psacc", bufs=1, space="PSUM") as psacc,
            tc.tile_pool(name="pstmp", bufs=1, space="PSUM") as pstmp,
        ):
            yres = singles.tile([128, NPAIR * 2 * W], f8, name="yres")

            # ---- input DMA: y on the scalar HWDGE queue (4 chunks), g on
            # the sync HWDGE queue (8 blocks) -> parallel issue + pacing.
            for c in range(4):
                lo = c * 16 * 2 * W
                hi = (c + 1) * 16 * 2 * W
                nc.scalar.dma_start(out=yres[:, lo:hi], in_=y_dram[:, lo:hi])

            gtiles = []
            for gb in range(2 * NBLK):
                g = gpool.tile([128, PPB * 2 * 128], f8)
                nc.sync.dma_start(out=g, in_=g_dram[gb * 128:(gb + 1) * 128, :])
                gtiles.append(g)

            # ---- PE clock warm-up (HAM ramp is ~3.4us of activity) ----
            wsrc = singles.tile([128, 128], f16)
            nc.gpsimd.memset(wsrc, 0.0)
            wps = pstmp.tile([128, 128], f32, tag="psT0", name="warmps")
            for _ in range(NWARM):
                nc.tensor.matmul(wps, wsrc, wsrc, start=True, stop=True)

            # ---- constants ----
            ones128 = singles.tile([128, 1], f32)
            nc.gpsimd.memset(ones128, 1.0)
            eps128 = singles.tile([128, 1], f32)
            nc.gpsimd.memset(eps128, EPS)
            ones16c = singles.tile([128, 1], f16)
            nc.gpsimd.memset(ones16c, 1.0)
            id16 = singles.tile([128, 128], f16)
            nc.gpsimd.memset(id16, 0.0)
            nc.gpsimd.affine_select(
                out=id16, in_=id16,
                compare_op=OP.not_equal,
                fill=1.0, base=0, pattern=[[-1, 128]], channel_multiplier=1,
            )
            ones_row = singles.tile([1, 128], f16)
            nc.gpsimd.memset(ones_row, 1.0)
            mhalf16 = singles.tile([128, 1], f16)
            nc.gpsimd.memset(mhalf16, -0.5)
            qneg_sb = singles.tile([1, 2 * 128], f16)
            # per-half feature indicator [4, 128] and cross-feature bias
            # -B*(1-ind); identical for both halves (local structure).
            ind16 = singles.tile([4, 128], f16)
            nc.gpsimd.memset(ind16, 0.0)
            i3 = ind16.rearrange("p (blk c) -> p blk c", c=32)
            nc.gpsimd.affine_select(
                out=i3, in_=i3, compare_op=OP.not_equal,
                fill=1.0, base=0, pattern=[[1, 4], [0, 32]],
                channel_multiplier=-1,
            )
            indB = singles.tile([4, 128], f16)
            nc.gpsimd.memset(indB, -BBIAS)
            b3 = indB.rearrange("p (blk c) -> p blk c", c=32)
            nc.gpsimd.affine_select(
                out=b3, in_=b3, compare_op=OP.not_equal,
                fill=0.0, base=0, pattern=[[1, 4], [0, 32]],
                channel_multiplier=-1,
            )
            # st cols: [wv0, wv1, eall0, eall1, rep0, rep1, rx0, rx1]
            st = ph2.tile([128, 8], f32)
            nc.gpsimd.memset(st, 0.0)

            # ---- preload the Exp table once; nothing else uses a table ----
            warm = ph2.tile([1, 2], f32)
            nc.scalar.activation(out=warm[0:1, 0:1], in_=ones128[0:1, 0:1], func=AF.Exp)

            # ---- phase 1: slab-ordered DoubleRow accumulation ----
            # ps[h][:, 0:128]=SWY_h, [:,128]=mass_raw, [:,129:131]=A_hi/lo
            ps = [psacc.tile([128, W], f32, name=f"acc{h}") for h in range(2)]

            # ---- per-half tail tiles ----
            mass = ph2.tile([128, 2], f32)
            inv = ph2.tile([128, 2], f32)
            a_ = ph2.tile([128, 2], f32)
            csq = ph2.tile([128, 2], f32)
            t1 = ph2.tile([128, 2], f32)
            cent16 = ph2.tile([128, 2 * 128], f16)
            ccT = ph2.tile([128, 2 * 128], f16)
            nshift = ph2.tile([128, 2], f32)
            sqc = scr.tile([128, 2 * 128], f16, tag="sqc")

            def emit_half_tail(h):
                hs = slice(h * 128, (h + 1) * 128)
                # critical chain: mass -> inv -> cent16 -> transpose -> ccT
                nc.vector.tensor_scalar_add(
                    mass[:, h:h + 1], in0=ps[h][:, 128:129], scalar1=eps128)
                nc.vector.reciprocal(inv[:, h:h + 1], mass[:, h:h + 1])
                with nc.allow_low_precision(reason="cent fp16 for exp terms"):
                    nc.scalar.activation(
                        out=cent16[:, hs], in_=ps[h][:, 0:128], func=AF.Copy,
                        scale=inv[:, h:h + 1])
                ps_t = pstmp.tile([128, 128], f32, tag=f"psT{h}", name=f"psT{h}")
                nc.tensor.matmul(ps_t, cent16[:, hs], id16, start=True, stop=True)
                nc.vector.tensor_scalar_mul(
                    nshift[:, h:h + 1], in0=ps_t[:, 0:1], scalar1=-CSC)
                with nc.allow_low_precision(reason="cc fp16 for exp terms"):
                    nc.scalar.activation(
                        out=ccT[:, hs], in_=ps_t,
                        func=AF.Identity, bias=nshift[:, h:h + 1], scale=CSC)
                sqf = scr.tile([128, 128], f16, tag=f"sqf{h}", name=f"sqf{h}")
                with nc.allow_low_precision(reason="scaled cc^2 fits fp16"):
                    nc.vector.tensor_mul(sqf, ccT[:, hs], ccT[:, hs])
                # repulsion operand: adjacent-column diffs of ccT
                dd = scr.tile([128, 127], f16, tag=f"dd{h}", name=f"dd{h}")
                with nc.allow_low_precision(reason="scaled cc diffs fp16"):
                    nc.vector.tensor_sub(dd, ccT[:, h * 128:h * 128 + 127],
                                         ccT[:, h * 128 + 1:(h + 1) * 128])
                    nc.vector.tensor_mul(dd, dd, dd)
                # ps_q = -q/2 directly (lhsT = -0.5 column)
                ps_q = pstmp.tile([1, 128], f32, tag="psq", name=f"psq{h}")
                nc.tensor.matmul(ps_q, mhalf16, sqf, start=True, stop=True)
                # dots + cross-feature bias
                pe = pstmp.tile([128, 128], f32, tag=f"psE{h}", name=f"psE{h}")
                nc.tensor.matmul(pe, ccT[:, hs], ccT[:, hs], start=True, stop=False)
                nc.tensor.matmul(pe, ind16, indB, start=False, stop=False)
                qn = qneg_sb[0:1, hs]
                with nc.allow_low_precision(reason="q fp16 rank-1 operand"):
                    nc.scalar.activation(out=qn, in_=ps_q, func=AF.Copy)
                ps_nd_t = pstmp.tile([1, 128], f32, tag="psq", name=f"psnd{h}")
                ps_nd = ps_nd_t[0:1, 0:127]
                nc.tensor.matmul(ps_nd, ones16c, dd, start=True, stop=True)
                nc.tensor.matmul(pe, ones_row, qn, start=False, stop=False)
                nc.tensor.matmul(pe, qn, ones_row, start=False, stop=True)
                # exps: repulsion first (input ready earliest), then pairwise
                en_row = ph2.tile([1, 127], f32, name=f"en{h}")
                nc.scalar.activation(out=en_row, in_=ps_nd, func=AF.Exp,
                                     scale=-1.0 / (CSC * CSC),
                                     accum_out=st[0:1, 4 + h:5 + h])
                e_full = scr.tile([128, 128], f16, tag=f"ef{h}", name=f"ef{h}")
                with nc.allow_low_precision(reason="E<=1 fp16; accum f32"):
                    nc.scalar.activation(out=e_full, in_=pe, func=AF.Exp,
                                         scale=2.0 / (CSC * CSC),
                                         accum_out=st[:, 2 + h:3 + h])
                # subtract the 3 feature-crossing pairs (local k = 31 mod 32)
                xview = en_row[0:1, 31:31 + 96].rearrange("p (m c) -> p m c", c=32)
                nc.vector.reduce_sum(st[0:1, 6 + h:7 + h], xview[:, :, 0:1],
                                     axis=mybir.AxisListType.XY)
                # off-critical stats
                nc.vector.reduce_sum(
                    a_[:, h:h + 1],
                    ps[h][:, 129:131].rearrange("p (one c) -> p one c", one=1),
                    axis=mybir.AxisListType.X)
                with nc.allow_low_precision(reason="csq via fp16 cent"):
                    nc.vector.tensor_mul(sqc[:, hs], cent16[:, hs], cent16[:, hs])
                nc.vector.reduce_sum(
                    csq[:, h:h + 1],
                    sqc[:, hs].rearrange("p (one c) -> p one c", one=1),
                    axis=mybir.AxisListType.X)
                nc.vector.tensor_mul(t1[:, h:h + 1], a_[:, h:h + 1], inv[:, h:h + 1])
                nc.vector.tensor_sub(st[:, h:h + 1], t1[:, h:h + 1], csq[:, h:h + 1])

            def emit_mm(u, h):
                blk, ul = divmod(u, PPB)
                g = gtiles[h * NBLK + blk]
                gv = g[:, ul * 256:(ul + 1) * 256].rearrange("p (i fk) -> p i fk", i=2)
                yv = yres[:, u * 2 * W:(u + 1) * 2 * W].rearrange(
                    "p (i w) -> p i w", i=2)
                nc.tensor.matmul(
                    ps[h], gv, yv,
                    start=(u == 0), stop=(u == NPAIR - 1), perf_mode=DR,
                )

            for u in range(NPAIR):
                emit_mm(u, 0)
            emit_half_tail(0)
            for u in range(NPAIR):
                emit_mm(u, 1)
            # bin_mass out early-ish on the scalar queue (host entropy)
            emit_half_tail(1)
            nc.scalar.dma_start(out=mass_dram, in_=mass)

            ps_res = pstmp.tile([1, 8], f32, tag="psres")
            nc.tensor.matmul(ps_res, ones128, st, start=True, stop=True)
            res = ph2.tile([1, 8], f32)
            nc.vector.tensor_copy(res, ps_res)
            nc.sync.dma_start(out=out_dram, in_=res)

    nc.compile()
    return nc


def get_nc():
    if "v5" not in _NC_CACHE:
        _NC_CACHE["v5"] = _build_nc()
    return _NC_CACHE["v5"]


def kernel(membership: np.ndarray, teacher_preds: np.ndarray, _trace: bool = False):
    from concourse.bass_utils import run_bass_kernel_spmd

    f8 = _f8dtype()
    m = np.asarray(membership, dtype=np.float32).reshape(N, F * K)
    y32 = np.asarray(teacher_preds, dtype=np.float32)
    ysq = np.einsum("nc,nc->n", y32, y32, dtype=np.float64).astype(np.float32)
    ysq_h = ysq.astype(f8)
    ysq_l = (ysq - ysq_h.astype(np.float32)).astype(f8)
    yslab = np.zeros((N, W), dtype=f8)
    yslab[:, 0:C] = y32.astype(f8)
    yslab[:, C] = np.float32(1.0)
    yslab[:, C + 1] = ysq_h
    yslab[:, C + 2] = ysq_l
    ypacked = _pack_y(yslab)

    m8 = m.astype(f8)
    nc = get_nc()
    in_maps = []
    for i in range(NCORES):
        in_maps.append({
            "g": _pack_g(m8[:, i * FK:(i + 1) * FK]),
            "y": ypacked,
        })
    res = run_bass_kernel_spmd(
        nc, in_maps, core_ids=list(range(NCORES)), trace=_trace,
    )
    parts = np.stack(
        [np.asarray(res.results[i]["out"][0], dtype=np.float64) for i in range(NCORES)]
    )
    masses = np.stack(
        [np.asarray(res.results[i]["mass"], dtype=np.float64) for i in range(NCORES)]
    )
    out = _finalize(parts, masses)
    if _trace:
        return out, res
    return out


if __name__ == "__main__":
    rng = np.random.default_rng(0)
    mem = rng.random((N, F, K), dtype=np.float32)
    tp = rng.random((N, C), dtype=np.float32)
    print(kernel(mem, tp))


# revision 4
# speedup vs baseline: 1.1798x; 1.1798x over previous
"""DispersionLoss kernel for Trainium2 (8 NeuronCores, Bass/Tile).

Reference computation (N=16384, F=64, K=32, C=128):
    bin_mass[f,k]  = sum_n m[n,f,k] + EPS
    SWY[f,k,c]     = sum_n m[n,f,k] * y[n,c]
    cent[f,k,c]    = SWY / bin_mass
    loss_dispersion= sum_fk (A/bin_mass - c_sq)   [EPS*c_sq/bin_mass ~1e-11, dropped]
        where A[f,k] = sum_n m[n,f,k]*|y_n|^2
    loss_entropy   = sum_fk p*log(p+EPS), p = bin_mass/N  (host, from shipped bin_mass)
    loss_repulsion = sum_f sum_k exp(-|cent[f,k]-cent[f,k+1]|^2)
    loss_inter     = sum_f (sum_{kj} exp(-pairwise) - K) / 2 / F

Sharding: over F (8 features per core) -> every loss term decomposes per-f,
no cross-core collectives; host sums 8 partial vectors.

v3 design (vs the 41.5us single-queue baseline):
  - inputs quantized to fp8 e4m3 on host; ysq shipped as fp8 hi+lo pair.
  - g repacked into TWO bin-half slabs (h=0: bins 0..127 = features 0..3,
    h=1: bins 128..255).  All of slab0 streams before slab1, so the h=0
    half finishes its PSUM accumulation ~2/3 through the DMA span and its
    FULL tail (centroids, pairwise exp, repulsion) runs overlapped with
    slab1's DMA + matmuls (tail PE ops interleaved between slab1 blocks).
    Post-DMA critical path = h=1 half-tail only.
  - input DMA split across both HWDGE queues: y (4 chunks) on the scalar
    queue, g (8 x 512KB blocks) on the sync queue -> ~380-400 GB/s
    aggregate (vs ~310 single-queue), earlier first byte.
  - no Ln on device: bin_mass (128,2) is DMA'd out (sync queue, off the
    scalar critical path) and the tiny (F,K) entropy term is computed on
    host in f64.  Scalar engine touches ONLY Copy/Identity/Exp -> the Exp
    table is loaded once, zero table reloads.
  - per-half pairwise (128x128, within-half; halves never share a feature)
    with the -B cross-feature block bias so exp() zeroes them and the Exp
    ACT's accum_out yields the inter-loss block sums for free.
  - HAM management: the PE clock-gate unlock needs ~3.4us of sustained
    activity and re-throttles when activity drops, so wide (512-col) junk
    matmuls fill every PE idle window: upfront ramp, slab0 inter-block
    gaps, and the h=1 tail.  Real DR matmuls then run at 2.4 GHz.
"""

import numpy as np

N = 16384
F = 64
K = 32
C = 128
NCORES = 8
F_PER_CORE = F // NCORES          # 8
FK = F_PER_CORE * K               # 256 bins per core
NPAIR = N // 256                  # 64 subtile pairs (DoubleRow: 256 rows/mm)
W = 132                           # moving cols: [y(128) | 1 | ysq_h | ysq_l | pad]
PPB = 16                          # pairs per g block
NBLK = NPAIR // PPB               # 4 blocks per slab
CSC = 16.0                        # centered-centroid scale (keeps fp16 normal)
BBIAS = 3840.0                    # cross-feature psE bias: exp arg -= 30
NWARM = 12                        # upfront wide junk MMs (~3.4us cold ramp)
JGAP = 8                          # wide junk MMs per slab0 inter-block gap

LAMBDA_ENTROPY = 0.1
LAMBDA_REPULSION = 0.5
LAMBDA_INTER = 0.3
EPS = 1e-8

_NC_CACHE = {}


def _f8dtype():
    import ml_dtypes
    return ml_dtypes.float8_e4m3


def _pack_g(gc: np.ndarray) -> np.ndarray:
    """(N, FK) fp8 -> (8*128, PPB*2*128): slab h (4 blocks), block row p holds,
    for the 16 pairs u of the block, [i=0 | i=1] x 128 half-bins where the
    n-row is 256*u + 128*i + p."""
    x = gc.reshape(NPAIR, 2, 128, 2, 128)           # u, i, p, h, fk
    x = x.reshape(NBLK, PPB, 2, 128, 2, 128)        # blk, ul, i, p, h, fk
    x = x.transpose(4, 0, 3, 1, 2, 5)               # h, blk, p, ul, i, fk
    return np.ascontiguousarray(x.reshape(2 * NBLK * 128, PPB * 2 * 128))


def _pack_y(yslab: np.ndarray) -> np.ndarray:
    """(N, W) fp8 -> (128, NPAIR*2*W): partition p holds pair-major slabs."""
    return np.ascontiguousarray(
        yslab.reshape(NPAIR, 2, 128, W).transpose(2, 0, 1, 3).reshape(128, NPAIR * 2 * W)
    )


def _finalize(parts: np.ndarray, masses: np.ndarray):
    """parts: (ncores, 8) = [wv0, wv1, eall0, eall1, rep0, rep1, rx0, rx1].
    masses: (ncores, 128, 2) = bin_mass (+EPS) per half."""
    r = parts.astype(np.float64).sum(axis=0)
    disp = r[0] + r[1]
    p = masses.astype(np.float64).reshape(-1) / N
    ent = float(np.sum(p * np.log(p + EPS)))
    rep = (r[4] + r[5]) - (r[6] + r[7])
    inter = (r[2] + r[3] - F * K) / (2.0 * F)
    tot = disp + LAMBDA_ENTROPY * ent + LAMBDA_REPULSION * rep + LAMBDA_INTER * inter
    return tuple(np.float32(v) for v in (tot, disp, ent, rep, inter))


def _build_nc():
    import concourse.bacc as bacc
    import concourse.tile as tile
    from concourse import mybir

    f32 = mybir.dt.float32
    f16 = mybir.dt.float16
    f8 = mybir.dt.float8e4
    DR = mybir.MatmulPerfMode.DoubleRow
    AF = mybir.ActivationFunctionType
    OP = mybir.AluOpType

    nc = bacc.Bacc("TRN2", target_bir_lowering=False, debug=False,
                   enable_asserts=False, enable_partition_id=False)
    g_dram = nc.dram_tensor("g", (2 * NBLK * 128, PPB * 2 * 128), f8,
                            kind="ExternalInput").ap()
    y_dram = nc.dram_tensor("y", (128, NPAIR * 2 * W), f8, kind="ExternalInput").ap()
    out_dram = nc.dram_tensor("out", (1, 8), f32, kind="ExternalOutput").ap()
    mass_dram = nc.dram_tensor("mass", (128, 2), f32, kind="ExternalOutput").ap()

    with tile.TileContext(nc) as tc:
        with (
            tc.tile_pool(name="singles", bufs=1) as singles,
            tc.tile_pool(name="gpool", bufs=8) as gpool,
            tc.tile_pool(name="scr", bufs=2) as scr,
            tc.tile_pool(name="ph2", bufs=1) as ph2,
            tc.tile_pool(name="psacc", bufs=1, space="PSUM") as psacc,
            tc.tile_pool(name="pstmp", bufs=1, space="PSUM") as pstmp,
        ):
            yres = singles.tile([128, NPAIR * 2 * W], f8, name="yres")

            # ---- input DMA: y on the scalar HWDGE queue (4 chunks), g on
            # the sync HWDGE queue (8 blocks) -> parallel issue + pacing.
            for c in range(4):
                lo = c * 16 * 2 * W
                hi = (c + 1) * 16 * 2 * W
                nc.scalar.dma_start(out=yres[:, lo:hi], in_=y_dram[:, lo:hi])

            gtiles = []
            for gb in range(2 * NBLK):
                g = gpool.tile([128, PPB * 2 * 128], f8)
                nc.sync.dma_start(out=g, in_=g_dram[gb * 128:(gb + 1) * 128, :])
                gtiles.append(g)

            # ---- PE junk source (wide: 512 moving cols ~ 213ns warm) ----
            wsrc = singles.tile([128, 512], f16)
            nc.gpsimd.memset(wsrc, 0.0)
            wps = pstmp.tile([128, 512], f32, tag="psT0", name="warmps")

            def emit_junk(n):
                for _ in range(n):
                    nc.tensor.matmul(wps, wsrc[:, 0:128], wsrc,
                                     start=True, stop=True)

            emit_junk(NWARM)

            # ---- constants ----
            ones128 = singles.tile([128, 1], f32)
            nc.gpsimd.memset(ones128, 1.0)
            eps128 = singles.tile([128, 1], f32)
            nc.gpsimd.memset(eps128, EPS)
            ones16c = singles.tile([128, 1], f16)
            nc.gpsimd.memset(ones16c, 1.0)
            id16 = singles.tile([128, 128], f16)
            nc.gpsimd.memset(id16, 0.0)
            nc.gpsimd.affine_select(
                out=id16, in_=id16,
                compare_op=OP.not_equal,
                fill=1.0, base=0, pattern=[[-1, 128]], channel_multiplier=1,
            )
            ones_row = singles.tile([1, 128], f16)
            nc.gpsimd.memset(ones_row, 1.0)
            mhalf16 = singles.tile([128, 1], f16)
            nc.gpsimd.memset(mhalf16, -0.5)
            qneg_sb = singles.tile([1, 2 * 128], f16)
            # per-half feature indicator [4, 128] and cross-feature bias
            # -B*(1-ind); identical for both halves (local structure).
            ind16 = singles.tile([4, 128], f16)
            nc.gpsimd.memset(ind16, 0.0)
            i3 = ind16.rearrange("p (blk c) -> p blk c", c=32)
            nc.gpsimd.affine_select(
                out=i3, in_=i3, compare_op=OP.not_equal,
                fill=1.0, base=0, pattern=[[1, 4], [0, 32]],
                channel_multiplier=-1,
            )
            indB = singles.tile([4, 128], f16)
            nc.gpsimd.memset(indB, -BBIAS)
            b3 = indB.rearrange("p (blk c) -> p blk c", c=32)
            nc.gpsimd.affine_select(
                out=b3, in_=b3, compare_op=OP.not_equal,
                fill=0.0, base=0, pattern=[[1, 4], [0, 32]],
                channel_multiplier=-1,
            )
            # st cols: [wv0, wv1, eall0, eall1, rep0, rep1, rx0, rx1]
            st = ph2.tile([128, 8], f32)
            nc.gpsimd.memset(st, 0.0)

            # ---- preload the Exp table once; nothing else uses a table ----
            warm = ph2.tile([1, 2], f32)
            nc.scalar.activation(out=warm[0:1, 0:1], in_=ones128[0:1, 0:1], func=AF.Exp)

            # ---- phase 1: slab-ordered DoubleRow accumulation ----
            # ps[h][:, 0:128]=SWY_h, [:,128]=mass_raw, [:,129:131]=A_hi/lo
            ps = [psacc.tile([128, W], f32, name=f"acc{h}") for h in range(2)]

            # ---- per-half tail tiles ----
            mass = ph2.tile([128, 2], f32)
            inv = ph2.tile([128, 2], f32)
            a_ = ph2.tile([128, 2], f32)
            csq = ph2.tile([128, 2], f32)
            t1 = ph2.tile([128, 2], f32)
            cent16 = ph2.tile([128, 2 * 128], f16)
            ccT = ph2.tile([128, 2 * 128], f16)
            nshift = ph2.tile([128, 2], f32)
            sqc = scr.tile([128, 2 * 128], f16, tag="sqc")

            class Tail:
                """Per-half tail, staged so its PE ops can interleave with
                the other half's matmul stream."""

                def __init__(self, h):
                    self.h = h
                    self.hs = slice(h * 128, (h + 1) * 128)

                def stage_a(self):
                    h = self.h
                    nc.vector.tensor_scalar_add(
                        mass[:, h:h + 1], in0=ps[h][:, 128:129], scalar1=eps128)
                    nc.vector.reciprocal(inv[:, h:h + 1], mass[:, h:h + 1])
                    with nc.allow_low_precision(reason="cent fp16 for exp"):
                        nc.scalar.activation(
                            out=cent16[:, self.hs], in_=ps[h][:, 0:128],
                            func=AF.Copy, scale=inv[:, h:h + 1])

                def stage_b(self):
                    h = self.h
                    self.ps_t = pstmp.tile([128, 128], f32, tag=f"psT{h}",
                                           name=f"psT{h}")
                    nc.tensor.matmul(self.ps_t, cent16[:, self.hs], id16,
                                     start=True, stop=True)
                    nc.vector.tensor_scalar_mul(
                        nshift[:, h:h + 1], in0=self.ps_t[:, 0:1], scalar1=-CSC)
                    with nc.allow_low_precision(reason="cc fp16 for exp"):
                        nc.scalar.activation(
                            out=ccT[:, self.hs], in_=self.ps_t,
                            func=AF.Identity, bias=nshift[:, h:h + 1], scale=CSC)

                def stage_c(self):
                    h = self.h
                    sqf = scr.tile([128, 128], f16, tag=f"sqf{h}", name=f"sqf{h}")
                    with nc.allow_low_precision(reason="scaled cc^2 fits fp16"):
                        nc.vector.tensor_mul(sqf, ccT[:, self.hs], ccT[:, self.hs])
                    dd = scr.tile([128, 127], f16, tag=f"dd{h}", name=f"dd{h}")
                    with nc.allow_low_precision(reason="scaled cc diffs fp16"):
                        nc.vector.tensor_sub(dd, ccT[:, h * 128:h * 128 + 127],
                                             ccT[:, h * 128 + 1:(h + 1) * 128])
                        nc.vector.tensor_mul(dd, dd, dd)
                    self.dd = dd
                    self.ps_q = pstmp.tile([1, 128], f32, tag="psq",
                                           name=f"psq{h}")
                    nc.tensor.matmul(self.ps_q, mhalf16, sqf, start=True, stop=True)
                    self.pe = pstmp.tile([128, 128], f32, tag=f"psE{h}",
                                         name=f"psE{h}")
                    nc.tensor.matmul(self.pe, ccT[:, self.hs], ccT[:, self.hs],
                                     start=True, stop=False)
                    nc.tensor.matmul(self.pe, ind16, indB, start=False, stop=False)

                def stage_d(self):
                    h = self.h
                    self.qn = qneg_sb[0:1, self.hs]
                    with nc.allow_low_precision(reason="q fp16 rank-1 operand"):
                        nc.scalar.activation(out=self.qn, in_=self.ps_q,
                                             func=AF.Copy)
                    ps_nd_t = pstmp.tile([1, 128], f32, tag="psq",
                                         name=f"psnd{h}")
                    self.ps_nd = ps_nd_t[0:1, 0:127]
                    nc.tensor.matmul(self.ps_nd, ones16c, self.dd,
                                     start=True, stop=True)
                    nc.tensor.matmul(self.pe, ones_row, self.qn,
                                     start=False, stop=False)
                    nc.tensor.matmul(self.pe, self.qn, ones_row,
                                     start=False, stop=True)

                def stage_e(self):
                    h = self.h
                    en_row = ph2.tile([1, 127], f32, name=f"en{h}")
                    nc.scalar.activation(out=en_row, in_=self.ps_nd, func=AF.Exp,
                                         scale=-1.0 / (CSC * CSC),
                                         accum_out=st[0:1, 4 + h:5 + h])
                    e_full = scr.tile([128, 128], f16, tag=f"ef{h}", name=f"ef{h}")
                    with nc.allow_low_precision(reason="E<=1 fp16; accum f32"):
                        nc.scalar.activation(out=e_full, in_=self.pe, func=AF.Exp,
                                             scale=2.0 / (CSC * CSC),
                                             accum_out=st[:, 2 + h:3 + h])
                    # subtract the 3 feature-crossing pairs (local k=31 mod 32)
                    xview = en_row[0:1, 31:31 + 96].rearrange(
                        "p (m c) -> p m c", c=32)
                    nc.vector.reduce_sum(st[0:1, 6 + h:7 + h], xview[:, :, 0:1],
                                         axis=mybir.AxisListType.XY)
                    # off-critical stats
                    nc.vector.reduce_sum(
                        a_[:, h:h + 1],
                        ps[h][:, 129:131].rearrange("p (one c) -> p one c", one=1),
                        axis=mybir.AxisListType.X)
                    with nc.allow_low_precision(reason="csq via fp16 cent"):
                        nc.vector.tensor_mul(sqc[:, self.hs], cent16[:, self.hs],
                                             cent16[:, self.hs])
                    nc.vector.reduce_sum(
                        csq[:, h:h + 1],
                        sqc[:, self.hs].rearrange("p (one c) -> p one c", one=1),
                        axis=mybir.AxisListType.X)
                    nc.vector.tensor_mul(t1[:, h:h + 1], a_[:, h:h + 1],
                                         inv[:, h:h + 1])
                    nc.vector.tensor_sub(st[:, h:h + 1], t1[:, h:h + 1],
                                         csq[:, h:h + 1])

            def emit_mm(u, h):
                blk, ul = divmod(u, PPB)
                g = gtiles[h * NBLK + blk]
                gv = g[:, ul * 256:(ul + 1) * 256].rearrange(
                    "p (i fk) -> p i fk", i=2)
                yv = yres[:, u * 2 * W:(u + 1) * 2 * W].rearrange(
                    "p (i w) -> p i w", i=2)
                nc.tensor.matmul(
                    ps[h], gv, yv,
                    start=(u == 0), stop=(u == NPAIR - 1), perf_mode=DR,
                )

            # slab0: junk fills the DMA pacing gaps between blocks
            for blk in range(NBLK):
                if blk:
                    emit_junk(JGAP)
                for ul in range(PPB):
                    emit_mm(blk * PPB + ul, 0)

            # h0 tail interleaved with slab1's matmul stream
            t0 = Tail(0)
            t0.stage_a()
            for ul in range(PPB):
                emit_mm(0 * PPB + ul, 1)
            t0.stage_b()
            for ul in range(PPB):
                emit_mm(1 * PPB + ul, 1)
            t0.stage_c()
            for ul in range(PPB):
                emit_mm(2 * PPB + ul, 1)
            t0.stage_d()
            for ul in range(PPB):
                emit_mm(3 * PPB + ul, 1)
            t0.stage_e()

            # h1 tail; junk keeps the PE clock unlocked through the chain
            t1_ = Tail(1)
            t1_.stage_a()
            nc.sync.dma_start(out=mass_dram, in_=mass)
            emit_junk(2)
            t1_.stage_b()
            emit_junk(2)
            t1_.stage_c()
            emit_junk(2)
            t1_.stage_d()
            emit_junk(2)
            t1_.stage_e()

            ps_res = pstmp.tile([1, 8], f32, tag="psres")
            nc.tensor.matmul(ps_res, ones128, st, start=True, stop=True)
            res = ph2.tile([1, 8], f32)
            nc.vector.tensor_copy(res, ps_res)
            nc.sync.dma_start(out=out_dram, in_=res)

    nc.compile()
    return nc


def get_nc():
    if "v6" not in _NC_CACHE:
        _NC_CACHE["v6"] = _build_nc()
    return _NC_CACHE["v6"]


def kernel(membership: np.ndarray, teacher_preds: np.ndarray, _trace: bool = False):
    from concourse.bass_utils import run_bass_kernel_spmd

    f8 = _f8dtype()
    m = np.asarray(membership, dtype=np.float32).reshape(N, F * K)
    y32 = np.asarray(teacher_preds, dtype=np.float32)
    ysq = np.einsum("nc,nc->n", y32, y32, dtype=np.float64).astype(np.float32)
    ysq_h = ysq.astype(f8)
    ysq_l = (ysq - ysq_h.astype(np.float32)).astype(f8)
    yslab = np.zeros((N, W), dtype=f8)
    yslab[:, 0:C] = y32.astype(f8)
    yslab[:, C] = np.float32(1.0)
    yslab[:, C + 1] = ysq_h
    yslab[:, C + 2] = ysq_l
    ypacked = _pack_y(yslab)

    m8 = m.astype(f8)
    nc = get_nc()
    in_maps = []
    for i in range(NCORES):
        in_maps.append({
            "g": _pack_g(m8[:, i * FK:(i + 1) * FK]),
            "y": ypacked,
        })
    res = run_bass_kernel_spmd(
        nc, in_maps, core_ids=list(range(NCORES)), trace=_trace,
    )
    parts = np.stack(
        [np.asarray(res.results[i]["out"][0], dtype=np.float64) for i in range(NCORES)]
    )
    masses = np.stack(
        [np.asarray(res.results[i]["mass"], dtype=np.float64) for i in range(NCORES)]
    )
    out = _finalize(parts, masses)
    if _trace:
        return out, res
    return out


if __name__ == "__main__":
    rng = np.random.default_rng(0)
    mem = rng.random((N, F, K), dtype=np.float32)
    tp = rng.random((N, C), dtype=np.float32)
    print(kernel(mem, tp))
